# revision 19
# baseline (speedup 1.0000x reference)
"""Trainium2 Bass kernel for nn_DistanceProbeAlternative (retrieval_knn).

Computes, per batch b:
    proj = emb[b] @ W.T                      # [S, R]
    dist[i, j] = ||proj_i||^2 - 2 proj_i . proj_j + ||proj_j||^2

Sharding: data-parallel over batch B=32 across 8 cores (4 batches/core).
W is replicated. No collectives.

v10 design:
  * Host lays out emb as embP16 [b, p, k, s]: input DMAs move 128
    partitions x multi-KB contiguous lines on the sync HWDGE ring
    (~420 B/ns). Batch 0 s-halved with per-half finish, batches 1-3
    k-halved, interleaved into the previous batch's dots stream.
  * Output PACKED [128, 4608] fp16 per batch, 3-4 contiguous DMAs;
    host unpacks + mirrors. All DMA on one ring, input first (FIFO
    priority).
  * PE warm-up dummies hold the HAM clock at 2.4GHz.
  * The proj_finish -> norms chain is pipelined in s-halves across the
    previous batch's dots hooks (i==3,4,5,6) so it never gates the PE.
  * Per i-tile: <=512 dots chunks into a deep 6-buf PSUM pool (shared
    with norm matmuls), drains (-2*d + n_i) on ACT (tiles 0-2) / DVE
    tensor_scalar (3-7), ONE wide fp16 rowrep add per tile on DVE
    (tiles 0-3) / GPSIMD (4-7). Last batch: drains ACT 0-4 / DVE 5-7,
    all adds on DVE (GPSIMD is slow and would stretch the tail).
"""

import numpy as np
from contextlib import ExitStack

import concourse.bass as bass
import concourse.bacc as bacc
import concourse.tile as tile
from concourse import mybir
from concourse.bass_utils import run_bass_kernel_spmd

B, S, D, R = 32, 1024, 1024, 128
NCORES = 8
BPC = B // NCORES
NDT = D // 128
NST = S // 128

F32 = mybir.dt.float32
F16 = mybir.dt.float16
IDENT = mybir.ActivationFunctionType.Identity
ADD = mybir.AluOpType.add
MULT = mybir.AluOpType.mult

WIDTHS = [S - 128 * i for i in range(NST)]
OFFS = [0]
for w in WIDTHS[:-1]:
    OFFS.append(OFFS[-1] + w)
TOT = OFFS[-1] + WIDTHS[-1]  # 4608

OUT_CUTS = [(1, 0, OFFS[2]), (3, OFFS[2], OFFS[4]), (7, OFFS[4], TOT)]
OUT_CUTS_LAST = [
    (1, 0, OFFS[2]), (3, OFFS[2], OFFS[4]),
    (5, OFFS[4], OFFS[6]), (7, OFFS[6], TOT),
]

N_WARM = 52


def build_nc():
    nc = bacc.Bacc("TRN2", target_bir_lowering=False, debug=False)

    embPd = nc.dram_tensor("embP16", [BPC, 128, NDT, S], F16, kind="ExternalInput")
    WTd = nc.dram_tensor("WT16", [128, D], F16, kind="ExternalInput")
    outPd = nc.dram_tensor("outP16", [BPC, 128, TOT], F16, kind="ExternalOutput")

    with tile.TileContext(nc) as tc, ExitStack() as ctx:
        constp = ctx.enter_context(tc.tile_pool(name="const", bufs=1))
        embT_p = ctx.enter_context(tc.tile_pool(name="embT", bufs=BPC))
        projT_p = ctx.enter_context(tc.tile_pool(name="projT", bufs=2))
        sq_p = ctx.enter_context(tc.tile_pool(name="sq", bufs=2))
        ncol_p = ctx.enter_context(tc.tile_pool(name="ncol", bufs=2))
        rowrep_p = ctx.enter_context(tc.tile_pool(name="rowrep", bufs=2))
        out_p = ctx.enter_context(tc.tile_pool(name="outsb", bufs=BPC))
        tmp_p = ctx.enter_context(tc.tile_pool(name="tmpsb", bufs=4))
        projps_p = ctx.enter_context(tc.tile_pool(name="projps", bufs=1, space="PSUM"))
        dotps_p = ctx.enter_context(tc.tile_pool(name="dotps", bufs=6, space="PSUM"))

        # W on the scalar (ACT) HWDGE ring: transfers in parallel with
        # batch 0's first chunk on the sync ring.
        WT16 = constp.tile([128, D], F16, name="WT16")
        nc.scalar.dma_start(out=WT16, in_=WTd.ap())

        ones16 = constp.tile([128, 128], F16, name="ones16")
        nc.vector.memset(ones16, 1.0)

        embTs = []
        for b in range(BPC):
            embT = embT_p.tile([128, NDT * S], F16, name="embT")
            embTs.append(embT)
            dst = embT.rearrange("p (k s) -> p k s", k=NDT)
            src = embPd.ap()[b]
            # stripe input chunks across both HWDGE rings
            if b == 0:
                for h in range(2):
                    sl = slice(512 * h, 512 * (h + 1))
                    eng = nc.sync if h == 0 else nc.scalar
                    eng.dma_start(out=dst[:, :, sl], in_=src[:, :, sl])
            else:
                for c in range(2):
                    ks = slice(4 * c, 4 * (c + 1))
                    eng = nc.sync if c == 0 else nc.scalar
                    eng.dma_start(out=dst[:, ks, :], in_=src[:, ks, :])

        warm_ps = dotps_p.tile([128, 128], F32, tag="dp", name="warm_ps")
        for _ in range(N_WARM):
            nc.tensor.matmul(warm_ps, ones16, ones16, start=True, stop=True)

        def proj_alloc():
            projT = projT_p.tile([128, S], F16, name="projT")
            sq = sq_p.tile([128, S], F16, name="sq")
            pps = projps_p.tile([128, 1024], F32, name="projps")
            return projT, sq, pps

        def proj_finish(tiles, h):
            projT, sq, pps = tiles
            sl = slice(512 * h, 512 * (h + 1))
            nc.scalar.copy(projT[:, sl], pps[:, sl])
            nc.vector.tensor_mul(sq[:, sl], projT[:, sl], projT[:, sl])

        def proj_s_half(b, tiles, h):
            embT = embTs[b]
            _, _, pps = tiles
            for k in range(NDT):
                nc.tensor.matmul(
                    pps[:, 512 * h : 512 * (h + 1)],
                    WT16[:, 128 * k : 128 * (k + 1)],
                    embT[:, S * k + 512 * h : S * k + 512 * (h + 1)],
                    start=(k == 0),
                    stop=(k == NDT - 1),
                )

        def proj_k_chunk(b, tiles, k0, k1):
            embT = embTs[b]
            _, _, pps = tiles
            for k in range(k0, k1):
                for h in range(2):
                    nc.tensor.matmul(
                        pps[:, 512 * h : 512 * (h + 1)],
                        WT16[:, 128 * k : 128 * (k + 1)],
                        embT[:, S * k + 512 * h : S * k + 512 * (h + 1)],
                        start=(k == 0),
                        stop=(k == NDT - 1),
                    )

        def norms_alloc():
            ncol = ncol_p.tile([128, 2 * NST], F32, name="ncol")
            rowrep = rowrep_p.tile([128, S], F16, name="rowrep")
            return ncol, rowrep

        def norms_h(sq, ncol, rowrep, h):
            """ncol cols for tiles 4h..4h+3 and rowrep s-half h."""
            ncol_ps = dotps_p.tile([128, 8], F32, tag="dp", name="ncol_ps")
            for t in range(4):
                i = 4 * h + t
                nc.tensor.matmul(
                    ncol_ps[:, 2 * t : 2 * t + 2],
                    sq[:, 128 * i : 128 * (i + 1)],
                    ones16[:, 0:2],
                    start=True,
                    stop=True,
                )
            nc.vector.tensor_copy(ncol[:, 8 * h : 8 * h + 8], ncol_ps)
            rp = dotps_p.tile([128, 512], F32, tag="dp", name="rp_ps")
            nc.tensor.matmul(
                rp, ones16, sq[:, 512 * h : 512 * (h + 1)],
                start=True, stop=True,
            )
            nc.scalar.copy(rowrep[:, 512 * h : 512 * (h + 1)], rp)

        def dots_tile(b, i, outsb, projT, ncol, rowrep, last):
            j0 = 128 * i
            Wi = WIDTHS[i]
            off = OFFS[i]
            nb = ncol[:, 2 * i : 2 * i + 1]
            tmp = tmp_p.tile([128, 1024], F16, name="tmp")[:, 0:Wi]
            drain_act = i <= 4 if last else i <= 2
            pos = 0
            while pos < Wi:
                w = min(512, Wi - pos)
                d_ps = dotps_p.tile([128, w], F32, tag="dp", name="d_ps")
                nc.tensor.matmul(
                    d_ps,
                    projT[:, j0 : j0 + 128],
                    projT[:, j0 + pos : j0 + pos + w],
                    start=True,
                    stop=True,
                )
                tc_ = tmp[:, pos : pos + w]
                if drain_act:
                    nc.scalar.activation(tc_, d_ps, IDENT, bias=nb, scale=-2.0)
                else:
                    nc.vector.tensor_scalar(tc_, d_ps, -2.0, nb, MULT, ADD)
                pos += w
            o = outsb[:, off : off + Wi]
            rr = rowrep[:, j0:S]
            if i >= 4 and not last:
                nc.gpsimd.tensor_add(o, tmp, rr)
            else:
                nc.vector.tensor_add(o, tmp, rr)

        # ---- main pipeline ----
        tiles = proj_alloc()
        proj_s_half(0, tiles, 0)
        proj_finish(tiles, 0)
        proj_s_half(0, tiles, 1)
        proj_finish(tiles, 1)
        norms = norms_alloc()
        norms_h(tiles[1], norms[0], norms[1], 0)
        norms_h(tiles[1], norms[0], norms[1], 1)

        for b in range(BPC):
            last = b + 1 >= BPC
            projT, sq, _ = tiles
            ncol, rowrep = norms
            outsb = out_p.tile([128, TOT], F16, name="outsb")
            cuts = OUT_CUTS_LAST if last else OUT_CUTS
            cut = 0
            for i in range(NST):
                dots_tile(b, i, outsb, projT, ncol, rowrep, last)
                if cut < len(cuts) and cuts[cut][0] == i:
                    _, c0, c1 = cuts[cut]
                    nc.sync.dma_start(
                        out=outPd.ap()[b, :, c0:c1], in_=outsb[:, c0:c1]
                    )
                    cut += 1
                if not last:
                    if i == 1:
                        tiles_n = proj_alloc()
                        proj_k_chunk(b + 1, tiles_n, 0, 4)
                    elif i == 3:
                        proj_k_chunk(b + 1, tiles_n, 4, NDT)
                        proj_finish(tiles_n, 0)
                    elif i == 4:
                        proj_finish(tiles_n, 1)
                        norms_n = norms_alloc()
                    elif i == 5:
                        norms_h(tiles_n[1], norms_n[0], norms_n[1], 0)
                    elif i == 6:
                        norms_h(tiles_n[1], norms_n[0], norms_n[1], 1)
            if not last:
                tiles = tiles_n
                norms = norms_n

    nc.finalize()
    return nc


_NC_CACHE = None


def _get_nc():
    global _NC_CACHE
    if _NC_CACHE is None:
        _NC_CACHE = build_nc()
    return _NC_CACHE


def _host_wt16(W):
    Wf = np.asarray(W, dtype=np.float32)
    wt = Wf.T.reshape(NDT, 128, 128).transpose(1, 0, 2).reshape(128, D)
    return np.ascontiguousarray(wt).astype(np.float16)


def _host_embp(emb16_core):
    return np.ascontiguousarray(
        emb16_core.reshape(BPC, S, NDT, 128).transpose(0, 3, 2, 1)
    )


def run(embeddings_batch, W, trace=False, tmpdir=None):
    nc = _get_nc()
    emb16 = np.asarray(embeddings_batch, dtype=np.float32).astype(np.float16)
    wt16 = _host_wt16(W)
    in_maps = [
        {
            "embP16": _host_embp(emb16[c * BPC : (c + 1) * BPC]),
            "WT16": wt16,
        }
        for c in range(NCORES)
    ]
    res = run_bass_kernel_spmd(
        nc, in_maps, core_ids=list(range(NCORES)), trace=trace, tmpdir=tmpdir
    )
    full = np.empty((B, S, S), dtype=np.float16)
    for c in range(NCORES):
        P = res.results[c]["outP16"]
        for b in range(BPC):
            g = c * BPC + b
            for i in range(NST):
                full[g, 128 * i : 128 * (i + 1), 128 * i : S] = P[
                    b, :, OFFS[i] : OFFS[i] + WIDTHS[i]
                ]
    NB = NST
    M = full.reshape(B, NB, 128, NB, 128)
    iu = np.triu_indices(NB, 1)
    M[:, iu[1], :, iu[0], :] = M[:, iu[0], :, iu[1], :].swapaxes(-1, -2)
    return full.astype(np.float32), res


def kernel(embeddings_batch, W):
    full, _ = run(embeddings_batch, W, trace=False)
    return full


# revision 20
# speedup vs baseline: 1.1048x; 1.1048x over previous
"""Trainium2 Bass kernel for nn_DistanceProbeAlternative (retrieval_knn).

Computes, per batch b:
    proj = emb[b] @ W.T                      # [S, R]
    dist[i, j] = ||proj_i||^2 - 2 proj_i . proj_j + ||proj_j||^2

Sharding: data-parallel over batch B=32 across 8 cores (4 batches/core).
W is replicated. No collectives.

v10 design:
  * Host lays out emb as embP16 [b, p, k, s]: input DMAs move 128
    partitions x multi-KB contiguous lines on the sync HWDGE ring
    (~420 B/ns). Batch 0 s-halved with per-half finish, batches 1-3
    k-halved, interleaved into the previous batch's dots stream.
  * Output PACKED [128, 4608] fp16 per batch, 3-4 contiguous DMAs;
    host unpacks + mirrors. All DMA on one ring, input first (FIFO
    priority).
  * PE warm-up dummies hold the HAM clock at 2.4GHz.
  * The proj_finish -> norms chain is pipelined in s-halves across the
    previous batch's dots hooks (i==3,4,5,6) so it never gates the PE.
  * Per i-tile: <=512 dots chunks into a deep 6-buf PSUM pool (shared
    with norm matmuls), drains (-2*d + n_i) on ACT (tiles 0-2) / DVE
    tensor_scalar (3-7), ONE wide fp16 rowrep add per tile on DVE
    (tiles 0-3) / GPSIMD (4-7). Last batch: drains ACT 0-4 / DVE 5-7,
    all adds on DVE (GPSIMD is slow and would stretch the tail).
"""

import numpy as np
from contextlib import ExitStack

import concourse.bass as bass
import concourse.bacc as bacc
import concourse.tile as tile
from concourse import mybir
from concourse.bass_utils import run_bass_kernel_spmd

B, S, D, R = 32, 1024, 1024, 128
NCORES = 8
BPC = B // NCORES
NDT = D // 128
NST = S // 128

F32 = mybir.dt.float32
F16 = mybir.dt.float16
IDENT = mybir.ActivationFunctionType.Identity
ADD = mybir.AluOpType.add
MULT = mybir.AluOpType.mult

WIDTHS = [S - 128 * i for i in range(NST)]
OFFS = [0]
for w in WIDTHS[:-1]:
    OFFS.append(OFFS[-1] + w)
TOT = OFFS[-1] + WIDTHS[-1]  # 4608

OUT_CUTS = [(1, 0, OFFS[2]), (3, OFFS[2], OFFS[4]), (7, OFFS[4], TOT)]
OUT_CUTS_LAST = [
    (1, 0, OFFS[2]), (3, OFFS[2], OFFS[4]),
    (5, OFFS[4], OFFS[6]), (6, OFFS[6], OFFS[7]), (7, OFFS[7], TOT),
]

N_WARM = 52


def build_nc():
    nc = bacc.Bacc("TRN2", target_bir_lowering=False, debug=False)

    embPd = nc.dram_tensor("embP16", [BPC, 128, NDT, S], F16, kind="ExternalInput")
    WTd = nc.dram_tensor("WT16", [128, D], F16, kind="ExternalInput")
    outPd = nc.dram_tensor("outP16", [BPC, 128, TOT], F16, kind="ExternalOutput")

    with tile.TileContext(nc) as tc, ExitStack() as ctx:
        constp = ctx.enter_context(tc.tile_pool(name="const", bufs=1))
        embT_p = ctx.enter_context(tc.tile_pool(name="embT", bufs=BPC))
        projT_p = ctx.enter_context(tc.tile_pool(name="projT", bufs=2))
        sq_p = ctx.enter_context(tc.tile_pool(name="sq", bufs=2))
        ncol_p = ctx.enter_context(tc.tile_pool(name="ncol", bufs=2))
        rowrep_p = ctx.enter_context(tc.tile_pool(name="rowrep", bufs=2))
        out_p = ctx.enter_context(tc.tile_pool(name="outsb", bufs=BPC))
        tmp_p = ctx.enter_context(tc.tile_pool(name="tmpsb", bufs=6))
        projps_p = ctx.enter_context(tc.tile_pool(name="projps", bufs=1, space="PSUM"))
        dotps_p = ctx.enter_context(tc.tile_pool(name="dotps", bufs=6, space="PSUM"))

        # W on the scalar (ACT) HWDGE ring: transfers in parallel with
        # batch 0's first chunk on the sync ring.
        WT16 = constp.tile([128, D], F16, name="WT16")
        nc.scalar.dma_start(out=WT16, in_=WTd.ap())

        ones16 = constp.tile([128, 128], F16, name="ones16")
        nc.vector.memset(ones16, 1.0)

        embTs = []
        for b in range(BPC):
            embT = embT_p.tile([128, NDT * S], F16, name="embT")
            embTs.append(embT)
            dst = embT.rearrange("p (k s) -> p k s", k=NDT)
            src = embPd.ap()[b]
            if b == 0:
                for h in range(2):
                    sl = slice(512 * h, 512 * (h + 1))
                    nc.sync.dma_start(out=dst[:, :, sl], in_=src[:, :, sl])
            else:
                for c in range(2):
                    ks = slice(4 * c, 4 * (c + 1))
                    nc.sync.dma_start(out=dst[:, ks, :], in_=src[:, ks, :])

        warm_ps = dotps_p.tile([128, 128], F32, tag="dp", name="warm_ps")
        for _ in range(N_WARM):
            nc.tensor.matmul(warm_ps, ones16, ones16, start=True, stop=True)

        def proj_alloc():
            projT = projT_p.tile([128, S], F16, name="projT")
            sq = sq_p.tile([128, S], F16, name="sq")
            pps = projps_p.tile([128, 1024], F32, name="projps")
            return projT, sq, pps

        def proj_finish(tiles, h):
            projT, sq, pps = tiles
            sl = slice(512 * h, 512 * (h + 1))
            nc.scalar.copy(projT[:, sl], pps[:, sl])
            nc.vector.tensor_mul(sq[:, sl], projT[:, sl], projT[:, sl])

        def proj_s_half(b, tiles, h):
            embT = embTs[b]
            _, _, pps = tiles
            for k in range(NDT):
                nc.tensor.matmul(
                    pps[:, 512 * h : 512 * (h + 1)],
                    WT16[:, 128 * k : 128 * (k + 1)],
                    embT[:, S * k + 512 * h : S * k + 512 * (h + 1)],
                    start=(k == 0),
                    stop=(k == NDT - 1),
                )

        def proj_k_chunk(b, tiles, k0, k1):
            embT = embTs[b]
            _, _, pps = tiles
            for k in range(k0, k1):
                for h in range(2):
                    nc.tensor.matmul(
                        pps[:, 512 * h : 512 * (h + 1)],
                        WT16[:, 128 * k : 128 * (k + 1)],
                        embT[:, S * k + 512 * h : S * k + 512 * (h + 1)],
                        start=(k == 0),
                        stop=(k == NDT - 1),
                    )

        def norms_alloc():
            ncol = ncol_p.tile([128, 2 * NST], F32, name="ncol")
            rowrep = rowrep_p.tile([128, S], F16, name="rowrep")
            return ncol, rowrep

        def norms_h(sq, ncol, rowrep, h):
            """ncol cols for tiles 4h..4h+3 and rowrep s-half h."""
            ncol_ps = dotps_p.tile([128, 8], F32, tag="dp", name="ncol_ps")
            for t in range(4):
                i = 4 * h + t
                nc.tensor.matmul(
                    ncol_ps[:, 2 * t : 2 * t + 2],
                    sq[:, 128 * i : 128 * (i + 1)],
                    ones16[:, 0:2],
                    start=True,
                    stop=True,
                )
            nc.vector.tensor_copy(ncol[:, 8 * h : 8 * h + 8], ncol_ps)
            rp = dotps_p.tile([128, 512], F32, tag="dp", name="rp_ps")
            nc.tensor.matmul(
                rp, ones16, sq[:, 512 * h : 512 * (h + 1)],
                start=True, stop=True,
            )
            nc.scalar.copy(rowrep[:, 512 * h : 512 * (h + 1)], rp)

        def dots_tile(b, i, outsb, projT, ncol, rowrep, last):
            j0 = 128 * i
            Wi = WIDTHS[i]
            off = OFFS[i]
            nb = ncol[:, 2 * i : 2 * i + 1]
            tmp = tmp_p.tile([128, 1024], F16, name="tmp")[:, 0:Wi]
            drain_act = i <= 4 if last else i <= 2
            pos = 0
            while pos < Wi:
                w = min(512, Wi - pos)
                d_ps = dotps_p.tile([128, w], F32, tag="dp", name="d_ps")
                nc.tensor.matmul(
                    d_ps,
                    projT[:, j0 : j0 + 128],
                    projT[:, j0 + pos : j0 + pos + w],
                    start=True,
                    stop=True,
                )
                tc_ = tmp[:, pos : pos + w]
                if drain_act:
                    nc.scalar.activation(tc_, d_ps, IDENT, bias=nb, scale=-2.0)
                else:
                    nc.vector.tensor_scalar(tc_, d_ps, -2.0, nb, MULT, ADD)
                pos += w
            o = outsb[:, off : off + Wi]
            rr = rowrep[:, j0:S]
            if i >= 4 and not last:
                nc.gpsimd.tensor_add(o, tmp, rr)
            else:
                nc.vector.tensor_add(o, tmp, rr)

        # ---- main pipeline ----
        tiles = proj_alloc()
        proj_s_half(0, tiles, 0)
        proj_finish(tiles, 0)
        proj_s_half(0, tiles, 1)
        proj_finish(tiles, 1)
        norms = norms_alloc()
        norms_h(tiles[1], norms[0], norms[1], 0)
        norms_h(tiles[1], norms[0], norms[1], 1)

        for b in range(BPC):
            last = b + 1 >= BPC
            projT, sq, _ = tiles
            ncol, rowrep = norms
            outsb = out_p.tile([128, TOT], F16, name="outsb")
            cuts = OUT_CUTS_LAST if last else OUT_CUTS
            cut = 0
            for i in range(NST):
                dots_tile(b, i, outsb, projT, ncol, rowrep, last)
                if cut < len(cuts) and cuts[cut][0] == i:
                    _, c0, c1 = cuts[cut]
                    nc.sync.dma_start(
                        out=outPd.ap()[b, :, c0:c1], in_=outsb[:, c0:c1]
                    )
                    cut += 1
                if not last:
                    if i == 1:
                        tiles_n = proj_alloc()
                        proj_k_chunk(b + 1, tiles_n, 0, 4)
                    elif i == 3:
                        proj_k_chunk(b + 1, tiles_n, 4, NDT)
                        proj_finish(tiles_n, 0)
                    elif i == 4:
                        proj_finish(tiles_n, 1)
                        norms_n = norms_alloc()
                    elif i == 5:
                        norms_h(tiles_n[1], norms_n[0], norms_n[1], 0)
                    elif i == 6:
                        norms_h(tiles_n[1], norms_n[0], norms_n[1], 1)
            if not last:
                tiles = tiles_n
                norms = norms_n

    nc.finalize()
    return nc


_NC_CACHE = None


def _get_nc():
    global _NC_CACHE
    if _NC_CACHE is None:
        _NC_CACHE = build_nc()
    return _NC_CACHE


def _host_wt16(W):
    Wf = np.asarray(W, dtype=np.float32)
    wt = Wf.T.reshape(NDT, 128, 128).transpose(1, 0, 2).reshape(128, D)
    return np.ascontiguousarray(wt).astype(np.float16)


def _host_embp(emb16_core):
    return np.ascontiguousarray(
        emb16_core.reshape(BPC, S, NDT, 128).transpose(0, 3, 2, 1)
    )


def run(embeddings_batch, W, trace=False, tmpdir=None):
    nc = _get_nc()
    emb16 = np.asarray(embeddings_batch, dtype=np.float32).astype(np.float16)
    wt16 = _host_wt16(W)
    in_maps = [
        {
            "embP16": _host_embp(emb16[c * BPC : (c + 1) * BPC]),
            "WT16": wt16,
        }
        for c in range(NCORES)
    ]
    res = run_bass_kernel_spmd(
        nc, in_maps, core_ids=list(range(NCORES)), trace=trace, tmpdir=tmpdir
    )
    full = np.empty((B, S, S), dtype=np.float16)
    for c in range(NCORES):
        P = res.results[c]["outP16"]
        for b in range(BPC):
            g = c * BPC + b
            for i in range(NST):
                full[g, 128 * i : 128 * (i + 1), 128 * i : S] = P[
                    b, :, OFFS[i] : OFFS[i] + WIDTHS[i]
                ]
    NB = NST
    M = full.reshape(B, NB, 128, NB, 128)
    iu = np.triu_indices(NB, 1)
    M[:, iu[1], :, iu[0], :] = M[:, iu[0], :, iu[1], :].swapaxes(-1, -2)
    return full.astype(np.float32), res


def kernel(embeddings_batch, W):
    full, _ = run(embeddings_batch, W, trace=False)
    return full


# revision 21
# speedup vs baseline: 1.1688x; 1.0580x over previous
"""Trainium2 Bass kernel for nn_DistanceProbeAlternative (retrieval_knn).

Computes, per batch b:
    proj = emb[b] @ W.T                      # [S, R]
    dist[i, j] = ||proj_i||^2 - 2 proj_i . proj_j + ||proj_j||^2

Sharding: data-parallel over batch B=32 across 8 cores (4 batches/core).
W is replicated. No collectives.

v10 design:
  * Host lays out emb as embP16 [b, p, k, s]: input DMAs move 128
    partitions x multi-KB contiguous lines on the sync HWDGE ring
    (~420 B/ns). Batch 0 s-halved with per-half finish, batches 1-3
    k-halved, interleaved into the previous batch's dots stream.
  * Output PACKED [128, 4608] fp16 per batch, 3-4 contiguous DMAs;
    host unpacks + mirrors. All DMA on one ring, input first (FIFO
    priority).
  * PE warm-up dummies hold the HAM clock at 2.4GHz.
  * The proj_finish -> norms chain is pipelined in s-halves across the
    previous batch's dots hooks (i==3,4,5,6) so it never gates the PE.
  * Per i-tile: <=512 dots chunks into a deep 6-buf PSUM pool (shared
    with norm matmuls), drains (-2*d + n_i) on ACT (tiles 0-2) / DVE
    tensor_scalar (3-7), ONE wide fp16 rowrep add per tile on DVE
    (tiles 0-3) / GPSIMD (4-7). Last batch: drains ACT 0-4 / DVE 5-7,
    all adds on DVE (GPSIMD is slow and would stretch the tail).
"""

import numpy as np
from contextlib import ExitStack

import concourse.bass as bass
import concourse.bacc as bacc
import concourse.tile as tile
from concourse import mybir
from concourse.bass_utils import run_bass_kernel_spmd

B, S, D, R = 32, 1024, 1024, 128
NCORES = 8
BPC = B // NCORES
NDT = D // 128
NST = S // 128

F32 = mybir.dt.float32
F16 = mybir.dt.float16
IDENT = mybir.ActivationFunctionType.Identity
ADD = mybir.AluOpType.add
MULT = mybir.AluOpType.mult

WIDTHS = [S - 128 * i for i in range(NST)]
OFFS = [0]
for w in WIDTHS[:-1]:
    OFFS.append(OFFS[-1] + w)
TOT = OFFS[-1] + WIDTHS[-1]  # 4608

OUT_CUTS = [(1, 0, OFFS[2]), (3, OFFS[2], OFFS[4]), (7, OFFS[4], TOT)]
OUT_CUTS_LAST = [
    (1, 0, OFFS[2]), (3, OFFS[2], OFFS[4]),
    (5, OFFS[4], OFFS[6]), (7, OFFS[6], TOT),
]

N_WARM = 52


def build_nc():
    nc = bacc.Bacc("TRN2", target_bir_lowering=False, debug=False)

    embPd = nc.dram_tensor("embP16", [BPC, 128, NDT, S], F16, kind="ExternalInput")
    WTd = nc.dram_tensor("WT16", [128, D], F16, kind="ExternalInput")
    outPd = nc.dram_tensor("outP16", [BPC, 128, TOT], F16, kind="ExternalOutput")

    with tile.TileContext(nc) as tc, ExitStack() as ctx:
        constp = ctx.enter_context(tc.tile_pool(name="const", bufs=1))
        embT_p = ctx.enter_context(tc.tile_pool(name="embT", bufs=BPC))
        projT_p = ctx.enter_context(tc.tile_pool(name="projT", bufs=2))
        sq_p = ctx.enter_context(tc.tile_pool(name="sq", bufs=2))
        ncol_p = ctx.enter_context(tc.tile_pool(name="ncol", bufs=2))
        rowrep_p = ctx.enter_context(tc.tile_pool(name="rowrep", bufs=2))
        out_p = ctx.enter_context(tc.tile_pool(name="outsb", bufs=BPC))
        tmp_p = ctx.enter_context(tc.tile_pool(name="tmpsb", bufs=4))
        projps_p = ctx.enter_context(tc.tile_pool(name="projps", bufs=1, space="PSUM"))
        dotps_p = ctx.enter_context(tc.tile_pool(name="dotps", bufs=6, space="PSUM"))

        # W on the scalar (ACT) HWDGE ring: transfers in parallel with
        # batch 0's first chunk on the sync ring.
        WT16 = constp.tile([128, D], F16, name="WT16")
        nc.scalar.dma_start(out=WT16, in_=WTd.ap())

        ones16 = constp.tile([128, 128], F16, name="ones16")
        nc.vector.memset(ones16, 1.0)

        embTs = []
        for b in range(BPC):
            embT = embT_p.tile([128, NDT * S], F16, name="embT")
            embTs.append(embT)
            dst = embT.rearrange("p (k s) -> p k s", k=NDT)
            src = embPd.ap()[b]
            if b == 0:
                for h in range(2):
                    sl = slice(512 * h, 512 * (h + 1))
                    nc.sync.dma_start(out=dst[:, :, sl], in_=src[:, :, sl])
            else:
                for c in range(2):
                    ks = slice(4 * c, 4 * (c + 1))
                    nc.sync.dma_start(out=dst[:, ks, :], in_=src[:, ks, :])

        warm_ps = dotps_p.tile([128, 128], F32, tag="dp", name="warm_ps")
        for _ in range(N_WARM):
            nc.tensor.matmul(warm_ps, ones16, ones16, start=True, stop=True)

        def proj_alloc():
            projT = projT_p.tile([128, S], F16, name="projT")
            sq = sq_p.tile([128, S], F16, name="sq")
            pps = projps_p.tile([128, 1024], F32, name="projps")
            return projT, sq, pps

        def proj_finish(tiles, h):
            projT, sq, pps = tiles
            sl = slice(512 * h, 512 * (h + 1))
            nc.scalar.copy(projT[:, sl], pps[:, sl])
            nc.vector.tensor_mul(sq[:, sl], projT[:, sl], projT[:, sl])

        def proj_s_half(b, tiles, h):
            embT = embTs[b]
            _, _, pps = tiles
            for k in range(NDT):
                nc.tensor.matmul(
                    pps[:, 512 * h : 512 * (h + 1)],
                    WT16[:, 128 * k : 128 * (k + 1)],
                    embT[:, S * k + 512 * h : S * k + 512 * (h + 1)],
                    start=(k == 0),
                    stop=(k == NDT - 1),
                )

        def proj_k_chunk(b, tiles, k0, k1):
            embT = embTs[b]
            _, _, pps = tiles
            for k in range(k0, k1):
                for h in range(2):
                    nc.tensor.matmul(
                        pps[:, 512 * h : 512 * (h + 1)],
                        WT16[:, 128 * k : 128 * (k + 1)],
                        embT[:, S * k + 512 * h : S * k + 512 * (h + 1)],
                        start=(k == 0),
                        stop=(k == NDT - 1),
                    )

        def norms_alloc():
            ncol = ncol_p.tile([128, 2 * NST], F32, name="ncol")
            rowrep = rowrep_p.tile([128, S], F16, name="rowrep")
            return ncol, rowrep

        def norms_h(sq, ncol, rowrep, h):
            """ncol cols for tiles 4h..4h+3 and rowrep s-half h."""
            ncol_ps = dotps_p.tile([128, 8], F32, tag="dp", name="ncol_ps")
            for t in range(4):
                i = 4 * h + t
                nc.tensor.matmul(
                    ncol_ps[:, 2 * t : 2 * t + 2],
                    sq[:, 128 * i : 128 * (i + 1)],
                    ones16[:, 0:2],
                    start=True,
                    stop=True,
                )
            nc.vector.tensor_copy(ncol[:, 8 * h : 8 * h + 8], ncol_ps)
            rp = dotps_p.tile([128, 512], F32, tag="dp", name="rp_ps")
            nc.tensor.matmul(
                rp, ones16, sq[:, 512 * h : 512 * (h + 1)],
                start=True, stop=True,
            )
            nc.scalar.copy(rowrep[:, 512 * h : 512 * (h + 1)], rp)

        def dots_tile(b, i, outsb, projT, ncol, rowrep, last):
            j0 = 128 * i
            Wi = WIDTHS[i]
            off = OFFS[i]
            nb = ncol[:, 2 * i : 2 * i + 1]
            tmp = tmp_p.tile([128, 1024], F16, name="tmp")[:, 0:Wi]
            drain_act = i <= 4 if last else i <= 2
            pos = 0
            while pos < Wi:
                w = min(512, Wi - pos)
                d_ps = dotps_p.tile([128, w], F32, tag="dp", name="d_ps")
                nc.tensor.matmul(
                    d_ps,
                    projT[:, j0 : j0 + 128],
                    projT[:, j0 + pos : j0 + pos + w],
                    start=True,
                    stop=True,
                )
                tc_ = tmp[:, pos : pos + w]
                if drain_act:
                    nc.scalar.activation(tc_, d_ps, IDENT, bias=nb, scale=-2.0)
                else:
                    nc.vector.tensor_scalar(tc_, d_ps, -2.0, nb, MULT, ADD)
                pos += w
            o = outsb[:, off : off + Wi]
            rr = rowrep[:, j0:S]
            if i >= 4 and not last:
                nc.gpsimd.tensor_add(o, tmp, rr)
            else:
                nc.vector.tensor_add(o, tmp, rr)

        # ---- main pipeline ----
        tiles = proj_alloc()
        proj_s_half(0, tiles, 0)
        proj_finish(tiles, 0)
        proj_s_half(0, tiles, 1)
        proj_finish(tiles, 1)
        norms = norms_alloc()
        norms_h(tiles[1], norms[0], norms[1], 0)
        norms_h(tiles[1], norms[0], norms[1], 1)

        for b in range(BPC):
            last = b + 1 >= BPC
            projT, sq, _ = tiles
            ncol, rowrep = norms
            outsb = out_p.tile([128, TOT], F16, name="outsb")
            cuts = OUT_CUTS_LAST if last else OUT_CUTS
            cut = 0
            for i in range(NST):
                dots_tile(b, i, outsb, projT, ncol, rowrep, last)
                if cut < len(cuts) and cuts[cut][0] == i:
                    _, c0, c1 = cuts[cut]
                    nc.sync.dma_start(
                        out=outPd.ap()[b, :, c0:c1], in_=outsb[:, c0:c1]
                    )
                    cut += 1
                if not last:
                    if i == 1:
                        tiles_n = proj_alloc()
                        proj_k_chunk(b + 1, tiles_n, 0, 4)
                    elif i == 3:
                        proj_k_chunk(b + 1, tiles_n, 4, NDT)
                        proj_finish(tiles_n, 0)
                    elif i == 4:
                        proj_finish(tiles_n, 1)
                        norms_n = norms_alloc()
                    elif i == 5:
                        norms_h(tiles_n[1], norms_n[0], norms_n[1], 0)
                    elif i == 6:
                        norms_h(tiles_n[1], norms_n[0], norms_n[1], 1)
            if not last:
                tiles = tiles_n
                norms = norms_n

    nc.finalize()
    return nc


_NC_CACHE = None


def _get_nc():
    global _NC_CACHE
    if _NC_CACHE is None:
        _NC_CACHE = build_nc()
    return _NC_CACHE


def _host_wt16(W):
    Wf = np.asarray(W, dtype=np.float32)
    wt = Wf.T.reshape(NDT, 128, 128).transpose(1, 0, 2).reshape(128, D)
    return np.ascontiguousarray(wt).astype(np.float16)


def _host_embp(emb16_core):
    return np.ascontiguousarray(
        emb16_core.reshape(BPC, S, NDT, 128).transpose(0, 3, 2, 1)
    )


def run(embeddings_batch, W, trace=False, tmpdir=None):
    nc = _get_nc()
    emb16 = np.asarray(embeddings_batch, dtype=np.float32).astype(np.float16)
    wt16 = _host_wt16(W)
    in_maps = [
        {
            "embP16": _host_embp(emb16[c * BPC : (c + 1) * BPC]),
            "WT16": wt16,
        }
        for c in range(NCORES)
    ]
    res = run_bass_kernel_spmd(
        nc, in_maps, core_ids=list(range(NCORES)), trace=trace, tmpdir=tmpdir
    )
    full = np.empty((B, S, S), dtype=np.float16)
    for c in range(NCORES):
        P = res.results[c]["outP16"]
        for b in range(BPC):
            g = c * BPC + b
            for i in range(NST):
                full[g, 128 * i : 128 * (i + 1), 128 * i : S] = P[
                    b, :, OFFS[i] : OFFS[i] + WIDTHS[i]
                ]
    NB = NST
    M = full.reshape(B, NB, 128, NB, 128)
    iu = np.triu_indices(NB, 1)
    M[:, iu[1], :, iu[0], :] = M[:, iu[0], :, iu[1], :].swapaxes(-1, -2)
    return full.astype(np.float32), res


def kernel(embeddings_batch, W):
    full, _ = run(embeddings_batch, W, trace=False)
    return full
